# revision 1
# baseline (speedup 1.0000x reference)
"""CTC loss (keras ctc_batch_cost semantics) on 8 Trainium2 NeuronCores.

Algorithm: linear-space CTC forward DP, reformulated as a *wavefront* over
extended-label lanes.  For each label lane k the whole time axis is computed
with one hardware linear-recurrence instruction (tensor_tensor_scan on the
DVE), so the serial chain is over k (129 steps), not over t (512 steps).

  E[k]_t = pb_t * (E[k]_{t-1} + O[k-1]_{t-1})                 (blank state 2k)
  O[k]_t = pl[k]_t * (O[k]_{t-1} + E[k]_{t-1} + kap_k*O[k-1]_{t-1})  (label 2k+1)

Probabilities are pre-scaled by 1/r_t with r_t = sum_s p_s^2 / sum_s p_s
(self-weighted mean over extended states) so the linear-space values stay
inside fp32 range for all 512 steps; the loss adds back sum_t log r_t.

The per-(b,t) gather y_pred[b,t,y_true[b,k]] is done as a one-hot matmul on
the tensor engine; the [k,t]->[b,t] re-layout is a flat SBUF->SBUF DMA.
Batch is sharded 32 per core (pure data parallelism).
"""

import sys

for _p in ("/opt/trn_rl_repo",):
    if _p not in sys.path:
        sys.path.insert(0, _p)

from contextlib import ExitStack

import numpy as np

import concourse.bacc as bacc
import concourse.bass as bass
import concourse.tile as tile
from concourse import mybir
from concourse.bass_utils import run_bass_kernel_spmd

F32 = mybir.dt.float32
AF = mybir.ActivationFunctionType
OP = mybir.AluOpType

B, T, C, L = 256, 512, 256, 128
NCORES = 8
BS = B // NCORES
EPS = 1e-7
BLANK = C - 1

_nc_cache = {}


def build_nc(bs=BS, t=T, c=C, l=L):
    key = (bs, t, c, l)
    if key in _nc_cache:
        return _nc_cache[key]
    CT = c // 128
    GRP = min(8, bs)
    nc = bacc.Bacc("TRN2")
    ypT = nc.declare_dram_parameter("ypT", [bs, c, t], F32, isOutput=False)
    Gd = nc.declare_dram_parameter("G", [bs, c, l], F32, isOutput=False)
    cntd = nc.declare_dram_parameter("cnt", [bs, c, 1], F32, isOutput=False)
    kapd = nc.declare_dram_parameter("kap", [bs, l], F32, isOutput=False)
    lossd = nc.declare_dram_parameter("loss", [bs, 1], F32, isOutput=True)

    with ExitStack() as ctx:
        tc = ctx.enter_context(tile.TileContext(nc))
        pers = ctx.enter_context(tc.tile_pool(name="pers", bufs=1))
        ypool = ctx.enter_context(tc.tile_pool(name="y", bufs=2))
        gpool = ctx.enter_context(tc.tile_pool(name="g", bufs=2))
        y2pool = ctx.enter_context(tc.tile_pool(name="y2", bufs=3))
        bcpool = ctx.enter_context(tc.tile_pool(name="bc", bufs=3))
        pspool = ctx.enter_context(
            tc.tile_pool(name="ps", bufs=3, space=bass.MemorySpace.PSUM)
        )
        psspool = ctx.enter_context(
            tc.tile_pool(name="pss", bufs=2, space=bass.MemorySpace.PSUM)
        )
        drampool = ctx.enter_context(
            tc.tile_pool(name="dram", bufs=2, space=bass.MemorySpace.DRAM)
        )

        pl_big = pers.tile([128, bs * t], F32)  # scaled gathered label probs
        PB = pers.tile([bs, t], F32)
        INVR = pers.tile([bs, t], F32)
        PBS = pers.tile([bs, t], F32)
        KAP = pers.tile([bs, l], F32)
        LOGACC = pers.tile([bs, 1], F32)
        SCR = pers.tile([bs, t], F32)
        ZERO = pers.tile([bs, t], F32)
        FIN = pers.tile([bs, 1], F32)
        LLOG = pers.tile([bs, 1], F32)
        LOSS = pers.tile([bs, 1], F32)

        nc.sync.dma_start(KAP[:], kapd[:])
        nc.gpsimd.memset(ZERO[:], 0.0)

        # ---------------- phase A: gather + scaling, in groups of GRP ----
        for g0 in range(0, bs, GRP):
            ng = min(GRP, bs - g0)
            ytiles = {}
            # A1: load y, blank rows, squares, s1/s2 matmuls
            i1 = bcpool.tile([GRP, t], F32, tag="i1")
            iv = bcpool.tile([GRP, t], F32, tag="iv")
            for loc in range(ng):
                b = g0 + loc
                psg = psspool.tile([33, t], F32, tag="psg")
                cts = []
                for ci in range(CT):
                    y = ypool.tile([128, t], F32, tag=f"Y{loc}_{ci}")
                    nc.sync.dma_start(y[:], ypT[b, ci * 128 : (ci + 1) * 128, :])
                    ytiles[(loc, ci)] = y
                    cn = gpool.tile([128, 1], F32, tag=f"cn{ci}")
                    nc.sync.dma_start(cn[:], cntd[b, ci * 128 : (ci + 1) * 128, :])
                    cts.append(cn)
                nc.sync.dma_start(PB[b : b + 1, :], ypT[b, BLANK : BLANK + 1, :])
                for ci in range(CT):
                    y2 = y2pool.tile([128, t], F32, tag="Y2")
                    nc.scalar.activation(y2[:], ytiles[(loc, ci)][:], AF.Square)
                    nc.tensor.matmul(
                        psg[0:1, :],
                        cts[ci][:],
                        ytiles[(loc, ci)][:],
                        start=(ci == 0),
                        stop=(ci == CT - 1),
                    )
                    nc.tensor.matmul(
                        psg[32:33, :],
                        cts[ci][:],
                        y2[:],
                        start=(ci == 0),
                        stop=(ci == CT - 1),
                    )
                # evac s1/s2 rows via SBUF bounce (engines can't start at
                # partition b; DMA can)
                pse = y2pool.tile([33, t], F32, tag="pse")
                nc.scalar.copy(pse[0:1, :], psg[0:1, :])
                nc.scalar.copy(pse[32:33, :], psg[32:33, :])
                nc.sync.dma_start(i1[loc : loc + 1, :], pse[0:1, :])
                nc.sync.dma_start(iv[loc : loc + 1, :], pse[32:33, :])
            # invr = s1 / s2   (r = s2/s1 = selfweighted mean prob)
            nc.vector.reciprocal(iv[0:ng, :], iv[0:ng, :])
            nc.vector.tensor_mul(iv[0:ng, :], iv[0:ng, :], i1[0:ng, :])
            nc.sync.dma_start(INVR[g0 : g0 + ng, :], iv[0:ng, :])
            # A2: gather matmul + scaled evac
            for loc in range(ng):
                b = g0 + loc
                gts = []
                for ci in range(CT):
                    gt = gpool.tile([128, l], F32, tag=f"G{ci}")
                    nc.sync.dma_start(gt[:], Gd[b, ci * 128 : (ci + 1) * 128, :])
                    gts.append(gt)
                ps = pspool.tile([128, t], F32, tag="plps")
                for ci in range(CT):
                    nc.tensor.matmul(
                        ps[0:l, :],
                        gts[ci][:],
                        ytiles[(loc, ci)][:],
                        start=(ci == 0),
                        stop=(ci == CT - 1),
                    )
                ivd = drampool.tile([1, t], F32, tag="ivd")
                nc.sync.dma_start(ivd[:], iv[loc : loc + 1, :])
                bc = bcpool.tile([128, t], F32, tag="bc")
                nc.sync.dma_start(bc[:], ivd[:].to_broadcast((128, t)))
                # pl_big[0:l, b] = (ps + EPS) * invr_bcast
                nc.vector.scalar_tensor_tensor(
                    pl_big[0:l, b * t : (b + 1) * t],
                    ps[0:l, :],
                    float(EPS),
                    bc[0:l, :],
                    OP.add,
                    OP.mult,
                )
        # scaled blank probs + log-accumulator
        nc.vector.scalar_tensor_tensor(
            PBS[:], PB[:], float(EPS), INVR[:], OP.add, OP.mult
        )
        nc.scalar.activation(SCR[:], INVR[:], AF.Ln, accum_out=LOGACC[:])

        # ---------------- phase B+C: wavefront over label lanes ----------
        E0 = pers.tile([bs, 1 + t], F32)
        Ebuf = [pers.tile([bs, 1 + t], F32, name=f"Eb{i}") for i in range(2)]
        Obuf = [pers.tile([bs, 1 + t], F32, name=f"Ob{i}") for i in range(3)]
        Dbuf = [pers.tile([bs, t], F32, name=f"Db{i}") for i in range(2)]
        plbuf = [pers.tile([bs, t], F32, name=f"plb{i}") for i in range(4)]
        nc.gpsimd.memset(E0[:, 0:1], 1.0)
        for tb in Ebuf + Obuf:
            nc.gpsimd.memset(tb[:, 0:1], 0.0)

        def shuffle(k, dst):
            # row k of pl_big, b-blocks -> [bs, t]
            nc.sync.dma_start(dst[:], pl_big[k : k + 1, :])

        # k = 0
        nc.vector.tensor_tensor_scan(
            E0[:, 1 : 1 + t], ZERO[:], PBS[:], E0[:, 0:1], OP.add, OP.mult
        )
        shuffle(0, plbuf[0])
        nc.vector.tensor_tensor_scan(
            Obuf[0][:, 1 : 1 + t],
            E0[:, 0:t],
            plbuf[0][:],
            Obuf[0][:, 0:1],
            OP.add,
            OP.mult,
        )
        prevO = Obuf[0]
        for k in range(1, l):
            Ek = Ebuf[k % 2]
            Ok = Obuf[k % 3]
            dl = Dbuf[k % 2]
            plk = plbuf[k % 4]
            shuffle(k, plk)
            nc.vector.tensor_tensor_scan(
                Ek[:, 1 : 1 + t], prevO[:, 0:t], PBS[:], Ek[:, 0:1], OP.add, OP.mult
            )
            nc.vector.scalar_tensor_tensor(
                dl[:], prevO[:, 0:t], KAP[:, k : k + 1], Ek[:, 0:t], OP.mult, OP.add
            )
            nc.vector.tensor_tensor_scan(
                Ok[:, 1 : 1 + t], dl[:], plk[:], Ok[:, 0:1], OP.add, OP.mult
            )
            prevO = Ok
        EL = Ebuf[l % 2]
        nc.vector.tensor_tensor_scan(
            EL[:, 1 : 1 + t], prevO[:, 0:t], PBS[:], EL[:, 0:1], OP.add, OP.mult
        )
        nc.vector.tensor_add(FIN[:], EL[:, t : t + 1], prevO[:, t : t + 1])
        nc.scalar.activation(LLOG[:], FIN[:], AF.Ln)
        nc.vector.tensor_sub(LOSS[:], LOGACC[:], LLOG[:])
        nc.sync.dma_start(lossd[:], LOSS[:])

    nc.finalize()
    _nc_cache[key] = nc
    return nc


def host_prep(y_true, y_pred, bs=BS, t=T, c=C, l=L):
    """Per-core input maps: transposed probs, one-hot gather matrix, counts,
    skip mask."""
    ncores = y_true.shape[0] // bs
    maps = []
    for core in range(ncores):
        sl = slice(core * bs, (core + 1) * bs)
        yt = np.asarray(y_true[sl], dtype=np.int32)
        ypT = np.ascontiguousarray(
            np.asarray(y_pred[sl], dtype=np.float32).transpose(0, 2, 1)
        )
        G = (yt[:, None, :] == np.arange(c, dtype=np.int32)[None, :, None]).astype(
            np.float32
        )
        cnt = G.sum(axis=2, keepdims=True)
        cnt[:, c - 1, 0] = l + 1.0  # blank multiplicity in extended states
        kap = np.zeros((bs, l), dtype=np.float32)
        kap[:, 1:] = (yt[:, 1:] != yt[:, :-1]).astype(np.float32)
        maps.append({"ypT": ypT, "G": G, "cnt": cnt, "kap": kap})
    return maps


def kernel(y_true, y_pred):
    nc = build_nc()
    maps = host_prep(y_true, y_pred)
    res = run_bass_kernel_spmd(nc, maps, list(range(NCORES)))
    loss = np.concatenate([res.results[i]["loss"] for i in range(NCORES)], axis=0)
    return loss.astype(np.float32)



# revision 2
# speedup vs baseline: 1.1344x; 1.1344x over previous
"""CTC loss (keras ctc_batch_cost semantics) on 8 Trainium2 NeuronCores.

Linear-space CTC forward DP as a wavefront over extended-label lanes: for
each label lane k the whole time axis is one hardware linear-recurrence
instruction (tensor_tensor_scan on the DVE), so the serial chain is over k
(128 steps), not t (512 steps).

  E[k]_t = pb_t * (E[k]_{t-1} + O[k-1]_{t-1})                 (blank state 2k)
  O[k]_t = pl[k]_t * (O[k]_{t-1} + E[k]_{t-1} + kap_k*O[k-1]_{t-1})

Probabilities are pre-scaled by invr_t = s1_t/s2_t (s1 = sum of extended-state
probs, s2 = sum of squares) to keep linear-space values in fp32 range; the
loss adds back sum_t log r_t.  The scaling is a gauge: it cancels exactly in
the final loss, so bf16 inputs / approx reciprocal do not hurt accuracy.

v2: bf16 datapath (halves DMA + DVE element traffic, 1-pass matmuls), packed
one-DMA-per-tensor input layout, s1/s2 accumulated across the batch in PSUM
via one-hot count columns in the gather weights, scaling broadcast via a
K=1 matmul instead of a DRAM round-trip.
"""

import sys

for _p in ("/opt/trn_rl_repo",):
    if _p not in sys.path:
        sys.path.insert(0, _p)

from contextlib import ExitStack

import ml_dtypes
import numpy as np

import concourse.bacc as bacc
import concourse.bass as bass
import concourse.tile as tile
from concourse import mybir
from concourse.bass_utils import run_bass_kernel_spmd

F32 = mybir.dt.float32
BF16 = mybir.dt.bfloat16
AF = mybir.ActivationFunctionType
OP = mybir.AluOpType
NPBF = np.dtype(ml_dtypes.bfloat16)

B, T, C, L = 256, 512, 256, 128
NCORES = 8
BS = B // NCORES
EPS = 1e-7
BLANK = C - 1

_nc_cache = {}


def build_nc(bs=BS, t=T, c=C, l=L):
    key = (bs, t, c, l)
    if key in _nc_cache:
        return _nc_cache[key]
    CT = c // 128
    nc = bacc.Bacc("TRN2")
    # ypk[b, c0, ci*t + tt] = y_pred[b, tt, ci*128 + c0]  (bf16)
    ypk = nc.declare_dram_parameter("ypk", [bs, 128, CT * t], BF16, isOutput=False)
    # gxk[b, c0, ci*(l+bs) + j] : j<l one-hot gather cols, j>=l cnt one-hot-col
    GW = l + bs
    gxk = nc.declare_dram_parameter("gxk", [bs, 128, CT * GW], BF16, isOutput=False)
    kapd = nc.declare_dram_parameter("kap", [bs, l], F32, isOutput=False)
    onesd = nc.declare_dram_parameter("ones", [1, 128], BF16, isOutput=False)
    lossd = nc.declare_dram_parameter("loss", [bs, 1], F32, isOutput=True)

    with ExitStack() as ctx:
        tc = ctx.enter_context(tile.TileContext(nc))
        pers = ctx.enter_context(tc.tile_pool(name="pers", bufs=1))
        ypool = ctx.enter_context(tc.tile_pool(name="y", bufs=3))
        y2pool = ctx.enter_context(tc.tile_pool(name="y2", bufs=3))
        gpool = ctx.enter_context(tc.tile_pool(name="g", bufs=3))
        rowpool = ctx.enter_context(tc.tile_pool(name="row", bufs=3))
        pspool = ctx.enter_context(
            tc.tile_pool(name="ps", bufs=3, space=bass.MemorySpace.PSUM)
        )
        psacc = ctx.enter_context(
            tc.tile_pool(name="psacc", bufs=1, space=bass.MemorySpace.PSUM)
        )
        psbc = ctx.enter_context(
            tc.tile_pool(name="psbc", bufs=2, space=bass.MemorySpace.PSUM)
        )

        pl_raw = pers.tile([128, bs * t], BF16)  # gathered label probs (raw)
        pl_big = pers.tile([128, bs * t], BF16)  # scaled label probs
        PB = pers.tile([bs, t], BF16)
        PBS = pers.tile([bs, t], BF16)
        INVR = pers.tile([bs, t], F32)
        INVR16 = pers.tile([bs, t], BF16)
        IV = pers.tile([bs, t], F32)
        SCR = pers.tile([bs, t], F32)
        KAP = pers.tile([bs, l], F32)
        ONES = pers.tile([1, 128], BF16)
        LOGACC = pers.tile([bs, 1], F32)
        ZERO = pers.tile([bs, t], BF16)
        FIN = pers.tile([bs, 1], F32)
        LLOG = pers.tile([bs, 1], F32)
        LOSS = pers.tile([bs, 1], F32)

        nc.sync.dma_start(KAP[:], kapd[:])
        nc.sync.dma_start(ONES[:], onesd[:])
        nc.gpsimd.memset(ZERO[:], 0.0)

        ps_s1 = psacc.tile([bs, t], F32)
        ps_s2 = psacc.tile([bs, t], F32)

        # ---------------- phase A: gather + s1/s2 ------------------------
        for b in range(bs):
            y = ypool.tile([128, CT * t], BF16, tag="y")
            nc.sync.dma_start(y[:], ypk[b])
            g = gpool.tile([128, CT * GW], BF16, tag="g")
            nc.sync.dma_start(g[:], gxk[b])
            # blank row: c=255 -> ci=1, c0=127
            nc.sync.dma_start(PB[b : b + 1, :], ypk[b, 127:128, t : 2 * t])
            y2 = y2pool.tile([128, CT * t], BF16, tag="y2")
            nc.scalar.activation(y2[:], y[:], AF.Square)
            psg = pspool.tile([128, t], F32, tag="plps")
            for ci in range(CT):
                nc.tensor.matmul(
                    psg[0:l, :],
                    g[:, ci * GW : ci * GW + l],
                    y[:, ci * t : (ci + 1) * t],
                    start=(ci == 0),
                    stop=(ci == CT - 1),
                )
            for ci in range(CT):
                nc.tensor.matmul(
                    ps_s1[:],
                    g[:, ci * GW + l : (ci + 1) * GW],
                    y[:, ci * t : (ci + 1) * t],
                    start=(b == 0 and ci == 0),
                    stop=(b == bs - 1 and ci == CT - 1),
                )
            for ci in range(CT):
                nc.tensor.matmul(
                    ps_s2[:],
                    g[:, ci * GW + l : (ci + 1) * GW],
                    y2[:, ci * t : (ci + 1) * t],
                    start=(b == 0 and ci == 0),
                    stop=(b == bs - 1 and ci == CT - 1),
                )
            nc.scalar.copy(pl_raw[:, b * t : (b + 1) * t], psg[0:l, :])

        # ---------------- tail: invr + scaled probs ----------------------
        nc.vector.reciprocal(IV[:], ps_s2[:])
        nc.vector.tensor_mul(INVR[:], IV[:], ps_s1[:])
        nc.scalar.activation(SCR[:], INVR[:], AF.Ln, accum_out=LOGACC[:])
        nc.vector.scalar_tensor_tensor(
            PBS[:], PB[:], float(EPS), INVR[:], OP.add, OP.mult
        )
        nc.scalar.copy(INVR16[:], INVR[:])
        for b in range(bs):
            ivrow = rowpool.tile([1, t], BF16, tag="ivrow")
            nc.sync.dma_start(ivrow[:], INVR16[b : b + 1, :])
            psb = psbc.tile([128, t], F32, tag="bc")
            nc.tensor.matmul(psb[:], ONES[:], ivrow[:], start=True, stop=True)
            nc.vector.scalar_tensor_tensor(
                pl_big[:, b * t : (b + 1) * t],
                pl_raw[:, b * t : (b + 1) * t],
                float(EPS),
                psb[:],
                OP.add,
                OP.mult,
            )

        # ---------------- phase B: wavefront over label lanes ------------
        E0 = pers.tile([bs, 1 + t], BF16)
        Ebuf = [pers.tile([bs, 1 + t], BF16, name=f"Eb{i}") for i in range(2)]
        Obuf = [pers.tile([bs, 1 + t], BF16, name=f"Ob{i}") for i in range(3)]
        Dbuf = [pers.tile([bs, t], BF16, name=f"Db{i}") for i in range(2)]
        plbuf = [pers.tile([bs, t], BF16, name=f"plb{i}") for i in range(4)]
        nc.gpsimd.memset(E0[:, 0:1], 1.0)
        for tb in Ebuf + Obuf:
            nc.gpsimd.memset(tb[:, 0:1], 0.0)

        def shuffle(k, dst):
            nc.sync.dma_start(dst[:], pl_big[k : k + 1, :])

        nc.vector.tensor_tensor_scan(
            E0[:, 1 : 1 + t], ZERO[:], PBS[:], E0[:, 0:1], OP.add, OP.mult
        )
        shuffle(0, plbuf[0])
        nc.vector.tensor_tensor_scan(
            Obuf[0][:, 1 : 1 + t],
            E0[:, 0:t],
            plbuf[0][:],
            Obuf[0][:, 0:1],
            OP.add,
            OP.mult,
        )
        prevO = Obuf[0]
        for k in range(1, l):
            Ek = Ebuf[k % 2]
            Ok = Obuf[k % 3]
            dl = Dbuf[k % 2]
            plk = plbuf[k % 4]
            shuffle(k, plk)
            nc.vector.tensor_tensor_scan(
                Ek[:, 1 : 1 + t], prevO[:, 0:t], PBS[:], Ek[:, 0:1], OP.add, OP.mult
            )
            nc.vector.scalar_tensor_tensor(
                dl[:], prevO[:, 0:t], KAP[:, k : k + 1], Ek[:, 0:t], OP.mult, OP.add
            )
            nc.vector.tensor_tensor_scan(
                Ok[:, 1 : 1 + t], dl[:], plk[:], Ok[:, 0:1], OP.add, OP.mult
            )
            prevO = Ok
        EL = Ebuf[l % 2]
        nc.vector.tensor_tensor_scan(
            EL[:, 1 : 1 + t], prevO[:, 0:t], PBS[:], EL[:, 0:1], OP.add, OP.mult
        )
        nc.vector.tensor_add(FIN[:], EL[:, t : t + 1], prevO[:, t : t + 1])
        nc.scalar.activation(LLOG[:], FIN[:], AF.Ln)
        nc.vector.tensor_sub(LOSS[:], LOGACC[:], LLOG[:])
        nc.sync.dma_start(lossd[:], LOSS[:])

    nc.finalize()
    _nc_cache[key] = nc
    return nc


def host_prep(y_true, y_pred, bs=BS, t=T, c=C, l=L):
    """Per-core packed inputs: bf16 transposed probs, bf16 gather weights with
    one-hot count columns, skip mask."""
    CT = c // 128
    GW = l + bs
    ncores = y_true.shape[0] // bs
    ones = np.ones((1, 128), dtype=NPBF)
    maps = []
    for core in range(ncores):
        sl = slice(core * bs, (core + 1) * bs)
        yt = np.asarray(y_true[sl], dtype=np.int32)
        # [bs, c, t] -> [bs, CT, 128, t] -> [bs, 128, CT*t]
        ypT = np.asarray(y_pred[sl], dtype=np.float32).transpose(0, 2, 1)
        ypk = np.ascontiguousarray(
            ypT.reshape(bs, CT, 128, t).transpose(0, 2, 1, 3).reshape(bs, 128, CT * t)
        ).astype(NPBF)
        G = (yt[:, None, :] == np.arange(c, dtype=np.int32)[None, :, None]).astype(
            np.float32
        )  # [bs, c, l]
        cnt = G.sum(axis=2)  # [bs, c]
        cnt[:, c - 1] = l + 1.0  # blank multiplicity in extended states
        gx = np.zeros((bs, c, GW), dtype=np.float32)
        gx[:, :, :l] = G
        for b in range(bs):
            gx[b, :, l + b] = cnt[b]
        gxk = np.ascontiguousarray(
            gx.reshape(bs, CT, 128, GW).transpose(0, 2, 1, 3).reshape(bs, 128, CT * GW)
        ).astype(NPBF)
        kap = np.zeros((bs, l), dtype=np.float32)
        kap[:, 1:] = (yt[:, 1:] != yt[:, :-1]).astype(np.float32)
        maps.append({"ypk": ypk, "gxk": gxk, "kap": kap, "ones": ones})
    return maps


def kernel(y_true, y_pred):
    nc = build_nc()
    maps = host_prep(y_true, y_pred)
    res = run_bass_kernel_spmd(nc, maps, list(range(NCORES)))
    loss = np.concatenate([res.results[i]["loss"] for i in range(NCORES)], axis=0)
    return loss.astype(np.float32)


# revision 3
# speedup vs baseline: 1.4203x; 1.2519x over previous
"""CTC loss (keras ctc_batch_cost semantics) on 8 Trainium2 NeuronCores.

Linear-space CTC forward DP as a wavefront over extended-label lanes: for
each label lane k the whole time axis is one hardware linear-recurrence
instruction (tensor_tensor_scan on the DVE), so the serial chain is over k
(128 steps), not t (512 steps).

Gauge-transformed recurrences (z_k = O_k * psi, psi_j = pb'_j, psi_T = 1):

  E[k]_j = pb'_j * E[k]_{j-1} + z[k-1]_{j-1}          (scan: mult-add)
  z[k]_j = (E[k]_j + z[k]_{j-1}) * Q[k]_{j-1}         (scan: add-mult)

with Q[k]_j = pl'[k]_j * pb'_{j+1} / pb'_j.  The skip-forbidden correction
(repeated labels) adds (kap-1)*z[k-1]_{j-1} to E[k]_j inside the z-scan's
in0; lanes where kap==1 for every batch row (on every core) skip that stt
entirely -- 2 DVE ops per lane instead of 3.

Probabilities are pre-scaled by invr_t = s1_t/s2_t (gauge, cancels exactly
in the loss; sum_t log r_t is added back).  Phase A runs in bf16 (DMA and
matmul halve); phase B runs in fp32 (bf16 is slower on the DVE scans).
"""

import sys

for _p in ("/opt/trn_rl_repo",):
    if _p not in sys.path:
        sys.path.insert(0, _p)

from contextlib import ExitStack

import ml_dtypes
import numpy as np

import concourse.bacc as bacc
import concourse.bass as bass
import concourse.tile as tile
from concourse import mybir
from concourse.bass_utils import run_bass_kernel_spmd

F32 = mybir.dt.float32
BF16 = mybir.dt.bfloat16
AF = mybir.ActivationFunctionType
OP = mybir.AluOpType
NPBF = np.dtype(ml_dtypes.bfloat16)

B, T, C, L = 256, 512, 256, 128
NCORES = 8
BS = B // NCORES
EPS = 1e-7
BLANK = C - 1

_nc_cache = {}


def build_nc(slow, bs=BS, t=T, c=C, l=L):
    """slow: tuple of lane indices (1..l-1) needing the kap correction stt."""
    key = (tuple(slow), bs, t, c, l)
    if key in _nc_cache:
        return _nc_cache[key]
    slowset = set(slow)
    CT = c // 128
    GW = l + bs
    nc = bacc.Bacc("TRN2")
    ypk = nc.declare_dram_parameter("ypk", [bs, 128, CT * t], BF16, isOutput=False)
    gxk = nc.declare_dram_parameter("gxk", [bs, 128, CT * GW], BF16, isOutput=False)
    km1d = nc.declare_dram_parameter("km1", [bs, l], F32, isOutput=False)
    onesd = nc.declare_dram_parameter("ones", [1, 128], BF16, isOutput=False)
    lossd = nc.declare_dram_parameter("loss", [bs, 1], F32, isOutput=True)

    with ExitStack() as ctx:
        tc = ctx.enter_context(tile.TileContext(nc))
        pers = ctx.enter_context(tc.tile_pool(name="pers", bufs=1))
        ypool = ctx.enter_context(tc.tile_pool(name="y", bufs=3))
        y2pool = ctx.enter_context(tc.tile_pool(name="y2", bufs=3))
        gpool = ctx.enter_context(tc.tile_pool(name="g", bufs=3))
        rowpool = ctx.enter_context(tc.tile_pool(name="row", bufs=3))
        pspool = ctx.enter_context(
            tc.tile_pool(name="ps", bufs=3, space=bass.MemorySpace.PSUM)
        )
        psacc = ctx.enter_context(
            tc.tile_pool(name="psacc", bufs=1, space=bass.MemorySpace.PSUM)
        )
        psbc = ctx.enter_context(
            tc.tile_pool(name="psbc", bufs=2, space=bass.MemorySpace.PSUM)
        )

        pl_raw = pers.tile([128, bs * t], BF16)  # gathered label probs (raw)
        pl_big = pers.tile([128, bs * t], F32)  # Q-scaled label probs
        PB = pers.tile([bs, t], BF16)
        PBS = pers.tile([bs, t], F32)
        PBSE = pers.tile([bs, t], F32)
        RP = pers.tile([bs, t], F32)
        RFAC = pers.tile([bs, t], F32)
        RFAC16 = pers.tile([bs, t], BF16)
        INVR = pers.tile([bs, t], F32)
        IV = pers.tile([bs, t], F32)
        SCR = pers.tile([bs, t], F32)
        KM1 = pers.tile([bs, l], F32)
        ONES = pers.tile([1, 128], BF16)
        LOGACC = pers.tile([bs, 1], F32)
        ZERO = pers.tile([bs, t], F32)
        FIN = pers.tile([bs, 1], F32)
        LLOG = pers.tile([bs, 1], F32)
        LOSS = pers.tile([bs, 1], F32)

        nc.sync.dma_start(KM1[:], km1d[:])
        nc.sync.dma_start(ONES[:], onesd[:])
        nc.gpsimd.memset(ZERO[:], 0.0)

        ps_s1 = psacc.tile([bs, t], F32)
        ps_s2 = psacc.tile([bs, t], F32)

        # ---------------- phase A: gather + s1/s2 ------------------------
        for b in range(bs):
            y = ypool.tile([128, CT * t], BF16, tag="y")
            nc.sync.dma_start(y[:], ypk[b])
            g = gpool.tile([128, CT * GW], BF16, tag="g")
            nc.sync.dma_start(g[:], gxk[b])
            # blank row: c=255 -> ci=1, c0=127
            nc.sync.dma_start(PB[b : b + 1, :], ypk[b, 127:128, t : 2 * t])
            y2 = y2pool.tile([128, CT * t], BF16, tag="y2")
            nc.scalar.activation(y2[:], y[:], AF.Square)
            psg = pspool.tile([128, t], F32, tag="plps")
            for ci in range(CT):
                nc.tensor.matmul(
                    psg[0:l, :],
                    g[:, ci * GW : ci * GW + l],
                    y[:, ci * t : (ci + 1) * t],
                    start=(ci == 0),
                    stop=(ci == CT - 1),
                )
            for ci in range(CT):
                nc.tensor.matmul(
                    ps_s1[:],
                    g[:, ci * GW + l : (ci + 1) * GW],
                    y[:, ci * t : (ci + 1) * t],
                    start=(b == 0 and ci == 0),
                    stop=(b == bs - 1 and ci == CT - 1),
                )
            for ci in range(CT):
                nc.tensor.matmul(
                    ps_s2[:],
                    g[:, ci * GW + l : (ci + 1) * GW],
                    y2[:, ci * t : (ci + 1) * t],
                    start=(b == 0 and ci == 0),
                    stop=(b == bs - 1 and ci == CT - 1),
                )
            nc.scalar.copy(pl_raw[:, b * t : (b + 1) * t], psg[0:l, :])

        # ---------------- tail: invr, gauge factors, scaled probs --------
        nc.vector.reciprocal(IV[:], ps_s2[:])
        nc.vector.tensor_mul(INVR[:], IV[:], ps_s1[:])
        nc.scalar.activation(SCR[:], INVR[:], AF.Ln, accum_out=LOGACC[:])
        nc.vector.scalar_tensor_tensor(
            PBS[:], PB[:], float(EPS), INVR[:], OP.add, OP.mult
        )
        # PBSE[j] = PBS[j+1], PBSE[T-1] = 1;  RFAC = INVR * PBSE / PBS
        nc.scalar.copy(PBSE[:, 0 : t - 1], PBS[:, 1:t])
        nc.gpsimd.memset(PBSE[:, t - 1 : t], 1.0)
        nc.vector.reciprocal(RP[:], PBS[:])
        nc.vector.tensor_mul(RFAC[:], RP[:], PBSE[:])
        nc.vector.tensor_mul(RFAC[:], RFAC[:], INVR[:])
        nc.scalar.copy(RFAC16[:], RFAC[:])
        for b in range(bs):
            ivrow = rowpool.tile([1, t], BF16, tag="ivrow")
            nc.sync.dma_start(ivrow[:], RFAC16[b : b + 1, :])
            psb = psbc.tile([128, t], F32, tag="bc")
            nc.tensor.matmul(psb[:], ONES[:], ivrow[:], start=True, stop=True)
            nc.vector.scalar_tensor_tensor(
                pl_big[:, b * t : (b + 1) * t],
                pl_raw[:, b * t : (b + 1) * t],
                float(EPS),
                psb[:],
                OP.add,
                OP.mult,
            )

        # ---------------- phase B: wavefront over label lanes ------------
        E0 = pers.tile([bs, 1 + t], F32)
        Ebuf = [pers.tile([bs, 1 + t], F32, name=f"Eb{i}") for i in range(2)]
        Zbuf = [pers.tile([bs, 1 + t], F32, name=f"Zb{i}") for i in range(3)]
        Dbuf = [pers.tile([bs, t], F32, name=f"Db{i}") for i in range(2)]
        plbuf = [pers.tile([bs, t], F32, name=f"plb{i}") for i in range(6)]
        nc.gpsimd.memset(E0[:, 0:1], 1.0)
        for tb in Ebuf + Zbuf:
            nc.gpsimd.memset(tb[:, 0:1], 0.0)

        def shuffle(k, dst):
            nc.sync.dma_start(dst[:], pl_big[k : k + 1, :])

        # lane 0: E0_j = pb'_j * E0_{j-1};  z0 = (E0_j + z_{j-1}) * Q0
        nc.vector.tensor_tensor_scan(
            E0[:, 1 : 1 + t], PBS[:], ZERO[:], E0[:, 0:1], OP.mult, OP.add
        )
        shuffle(0, plbuf[0])
        nc.vector.tensor_tensor_scan(
            Zbuf[0][:, 1 : 1 + t],
            E0[:, 1 : 1 + t],
            plbuf[0][:],
            Zbuf[0][:, 0:1],
            OP.add,
            OP.mult,
        )
        prevZ = Zbuf[0]
        for k in range(1, l):
            Ek = Ebuf[k % 2]
            Zk = Zbuf[k % 3]
            plk = plbuf[k % 6]
            shuffle(k, plk)
            nc.vector.tensor_tensor_scan(
                Ek[:, 1 : 1 + t], PBS[:], prevZ[:, 0:t], Ek[:, 0:1], OP.mult, OP.add
            )
            if k in slowset:
                dl = Dbuf[k % 2]
                nc.vector.scalar_tensor_tensor(
                    dl[:],
                    prevZ[:, 0:t],
                    KM1[:, k : k + 1],
                    Ek[:, 1 : 1 + t],
                    OP.mult,
                    OP.add,
                )
                zin = dl[:]
            else:
                zin = Ek[:, 1 : 1 + t]
            nc.vector.tensor_tensor_scan(
                Zk[:, 1 : 1 + t], zin, plk[:], Zk[:, 0:1], OP.add, OP.mult
            )
            prevZ = Zk
        EL = Ebuf[l % 2]
        nc.vector.tensor_tensor_scan(
            EL[:, 1 : 1 + t], PBS[:], prevZ[:, 0:t], EL[:, 0:1], OP.mult, OP.add
        )
        nc.vector.tensor_add(FIN[:], EL[:, t : t + 1], prevZ[:, t : t + 1])
        nc.scalar.activation(LLOG[:], FIN[:], AF.Ln)
        nc.vector.tensor_sub(LOSS[:], LOGACC[:], LLOG[:])
        nc.sync.dma_start(lossd[:], LOSS[:])

    nc.finalize()
    _nc_cache[key] = nc
    return nc


def lane_flags(y_true, l=L):
    """Lanes needing the kap stt: any repeated label at position k across the
    FULL batch (one SPMD program serves all cores)."""
    yt = np.asarray(y_true, dtype=np.int32)
    rep = yt[:, 1:] == yt[:, :-1]  # [B, l-1]
    return tuple(int(k) for k in range(1, l) if rep[:, k - 1].any())


def host_prep(y_true, y_pred, bs=BS, t=T, c=C, l=L):
    """Per-core packed inputs: bf16 transposed probs, bf16 gather weights with
    one-hot count columns, (kap-1) mask."""
    CT = c // 128
    GW = l + bs
    ncores = y_true.shape[0] // bs
    ones = np.ones((1, 128), dtype=NPBF)
    maps = []
    for core in range(ncores):
        sl = slice(core * bs, (core + 1) * bs)
        yt = np.asarray(y_true[sl], dtype=np.int32)
        ypT = np.asarray(y_pred[sl], dtype=np.float32).transpose(0, 2, 1)
        ypk = np.ascontiguousarray(
            ypT.reshape(bs, CT, 128, t).transpose(0, 2, 1, 3).reshape(bs, 128, CT * t)
        ).astype(NPBF)
        G = (yt[:, None, :] == np.arange(c, dtype=np.int32)[None, :, None]).astype(
            np.float32
        )
        cnt = G.sum(axis=2)
        cnt[:, c - 1] = l + 1.0  # blank multiplicity in extended states
        gx = np.zeros((bs, c, GW), dtype=np.float32)
        gx[:, :, :l] = G
        for b in range(bs):
            gx[b, :, l + b] = cnt[b]
        gxk = np.ascontiguousarray(
            gx.reshape(bs, CT, 128, GW).transpose(0, 2, 1, 3).reshape(bs, 128, CT * GW)
        ).astype(NPBF)
        kap = np.zeros((bs, l), dtype=np.float32)
        kap[:, 1:] = (yt[:, 1:] != yt[:, :-1]).astype(np.float32)
        maps.append({"ypk": ypk, "gxk": gxk, "km1": kap - 1.0, "ones": ones})
    return maps


def kernel(y_true, y_pred):
    nc = build_nc(lane_flags(y_true))
    maps = host_prep(y_true, y_pred)
    res = run_bass_kernel_spmd(nc, maps, list(range(NCORES)))
    loss = np.concatenate([res.results[i]["loss"] for i in range(NCORES)], axis=0)
    return loss.astype(np.float32)


# revision 10
# speedup vs baseline: 1.5567x; 1.0961x over previous
"""CTC loss (keras ctc_batch_cost semantics) on 8 Trainium2 NeuronCores.

Linear-space CTC forward DP as a wavefront over extended-label lanes: for
each label lane k the whole time axis is one hardware linear-recurrence
instruction (tensor_tensor_scan on the DVE), so the serial chain is over k
(128 steps), not t (512 steps).

Gauge-transformed recurrences (z_k = O_k * psi, psi_j = pb'_j, psi_T = 1):

  E[k]_j = pb'_j * E[k]_{j-1} + z[k-1]_{j-1}          (scan: mult-add)
  z[k]_j = (E[k]_j + z[k]_{j-1}) * Q[k]_{j-1}         (scan: add-mult)

with Q[k]_j = pl'[k]_j * pb'_{j+1} / pb'_j.  The skip-forbidden correction
(repeated labels) adds (kap-1)*z[k-1]_{j-1} inside the z-scan's in0; lanes
where kap==1 for every batch row skip that stt -- 2 DVE ops per lane.

Windowed scans: lane k's alpha is 0 for t < k (not enough emissions) and
irrelevant for t > T-L+1+k (cannot reach the final states), so each scan
covers a 386..387-step window instead of 512 -- exact, not approximate.

Probabilities are pre-scaled by invr_t = s1_t/s2_t (gauge, cancels exactly
in the loss; sum_t log r_t is added back).  Phase A runs in bf16; phase B
in fp32 (bf16 is slower on the DVE scans).
"""

import sys

for _p in ("/opt/trn_rl_repo",):
    if _p not in sys.path:
        sys.path.insert(0, _p)

from contextlib import ExitStack

import ml_dtypes
import numpy as np

import concourse.bacc as bacc
import concourse.bass as bass
import concourse.tile as tile
from concourse import mybir
from concourse.bass_utils import run_bass_kernel_spmd

F32 = mybir.dt.float32
BF16 = mybir.dt.bfloat16
AF = mybir.ActivationFunctionType
OP = mybir.AluOpType
NPBF = np.dtype(ml_dtypes.bfloat16)

B, T, C, L = 256, 512, 256, 128
NCORES = 8
BS = B // NCORES
EPS = 1e-7
BLANK = C - 1

_nc_cache = {}


def build_nc(slow, bs=BS, t=T, c=C, l=L):
    """slow: tuple of lane indices (1..l-1) needing the kap correction stt."""
    key = (tuple(slow), bs, t, c, l)
    if key in _nc_cache:
        return _nc_cache[key]
    slowset = set(slow)
    CT = c // 128
    GW = l + bs
    W = t - l + 2  # scan window length (386)
    nc = bacc.Bacc("TRN2")
    ypk = nc.declare_dram_parameter("ypk", [bs, 128, CT * t], BF16, isOutput=False)
    gxk = nc.declare_dram_parameter("gxk", [bs, 128, CT * GW], BF16, isOutput=False)
    km1d = nc.declare_dram_parameter("km1", [bs, l], F32, isOutput=False)
    onesd = nc.declare_dram_parameter("ones", [1, 128], BF16, isOutput=False)
    lossd = nc.declare_dram_parameter("loss", [bs, 1], F32, isOutput=True)

    with ExitStack() as ctx:
        tc = ctx.enter_context(tile.TileContext(nc))
        pers = ctx.enter_context(tc.tile_pool(name="pers", bufs=1))
        ypool = ctx.enter_context(tc.tile_pool(name="y", bufs=3))
        y2pool = ctx.enter_context(tc.tile_pool(name="y2", bufs=3))
        gpool = ctx.enter_context(tc.tile_pool(name="g", bufs=3))
        rowpool = ctx.enter_context(tc.tile_pool(name="row", bufs=3))
        pspool = ctx.enter_context(
            tc.tile_pool(name="ps", bufs=3, space=bass.MemorySpace.PSUM)
        )
        psacc = ctx.enter_context(
            tc.tile_pool(name="psacc", bufs=1, space=bass.MemorySpace.PSUM)
        )
        psbc = ctx.enter_context(
            tc.tile_pool(name="psbc", bufs=2, space=bass.MemorySpace.PSUM)
        )

        pl_raw = pers.tile([128, bs * t], BF16)
        pl_big = pers.tile([128, bs * t], F32)
        PB = pers.tile([bs, t], BF16)
        PBe = pers.tile([bs, t], F32)
        PBR = pers.tile([bs, t], F32)
        PBSH = pers.tile([bs, t], F32)
        PRE1 = pers.tile([bs, t], F32)
        INVSH = pers.tile([bs, t], F32)
        PBS = pers.tile([bs, t], F32)
        RFAC = pers.tile([bs, t], F32)
        RFAC16 = pers.tile([bs, t], BF16)
        INVR = pers.tile([bs, t], F32)
        IV = pers.tile([bs, t], F32)
        SCR = pers.tile([bs, t], F32)
        KM1 = pers.tile([bs, l], F32)
        ONES = pers.tile([1, 128], BF16)
        LOGACC = pers.tile([bs, 1], F32)
        ZERO = pers.tile([bs, t], F32)
        FIN = pers.tile([bs, 1], F32)
        LLOG = pers.tile([bs, 1], F32)
        LOSS = pers.tile([bs, 1], F32)

        nc.sync.dma_start(KM1[:], km1d[:])
        nc.sync.dma_start(ONES[:], onesd[:])
        nc.gpsimd.memset(ZERO[:], 0.0)

        ps_s1 = psacc.tile([bs, t], F32)
        ps_s2 = psacc.tile([bs, t], F32)

        # ---------------- phase A: gather + s1/s2 ------------------------
        for b in range(bs):
            y = ypool.tile([128, CT * t], BF16, tag="y")
            nc.sync.dma_start(y[:], ypk[b])
            g = gpool.tile([128, CT * GW], BF16, tag="g")
            nc.gpsimd.dma_start(g[:], gxk[b])
            nc.gpsimd.dma_start(PB[b : b + 1, :], ypk[b, 127:128, t : 2 * t])
            y2 = y2pool.tile([128, CT * t], BF16, tag="y2")
            nc.scalar.activation(y2[:], y[:], AF.Square)
            psg = pspool.tile([128, t], F32, tag="plps")
            for ci in range(CT):
                nc.tensor.matmul(
                    psg[0:l, :],
                    g[:, ci * GW : ci * GW + l],
                    y[:, ci * t : (ci + 1) * t],
                    start=(ci == 0),
                    stop=(ci == CT - 1),
                )
            for ci in range(CT):
                nc.tensor.matmul(
                    ps_s1[:],
                    g[:, ci * GW + l : (ci + 1) * GW],
                    y[:, ci * t : (ci + 1) * t],
                    start=(b == 0 and ci == 0),
                    stop=(b == bs - 1 and ci == CT - 1),
                )
            for ci in range(CT):
                nc.tensor.matmul(
                    ps_s2[:],
                    g[:, ci * GW + l : (ci + 1) * GW],
                    y2[:, ci * t : (ci + 1) * t],
                    start=(b == 0 and ci == 0),
                    stop=(b == bs - 1 and ci == CT - 1),
                )
            nc.scalar.copy(pl_raw[:, b * t : (b + 1) * t], psg[0:l, :])

        # pb-derived gauge factors (PB rows complete mid-loop; overlaps A)
        nc.vector.scalar_tensor_tensor(
            PBe[:], PB[:], float(EPS), ZERO[:], OP.add, OP.add
        )
        nc.vector.reciprocal(PBR[:], PBe[:])
        nc.scalar.copy(PBSH[:, 0 : t - 1], PBe[:, 1:t])
        nc.gpsimd.memset(PBSH[:, t - 1 : t], 1.0)
        nc.vector.tensor_mul(PRE1[:], PBSH[:], PBR[:])

        # ---------------- tail: invr, gauge factors, scaled probs --------
        nc.vector.reciprocal(IV[:], ps_s2[:])
        nc.vector.tensor_mul(INVR[:], IV[:], ps_s1[:])
        nc.vector.tensor_mul(PBS[:], PBe[:], INVR[:])
        nc.scalar.activation(SCR[:], INVR[:], AF.Ln, accum_out=LOGACC[:])
        nc.scalar.copy(INVSH[:, 0 : t - 1], INVR[:, 1:t])
        nc.gpsimd.memset(INVSH[:, t - 1 : t], 1.0)
        nc.vector.tensor_mul(RFAC[:], PRE1[:], INVSH[:])
        nc.scalar.copy(RFAC16[:], RFAC[:])
        for b in range(bs):
            ivrow = rowpool.tile([1, t], BF16, tag="ivrow")
            nc.gpsimd.dma_start(ivrow[:], RFAC16[b : b + 1, :])
            psb = psbc.tile([128, t], F32, tag="bc")
            nc.tensor.matmul(psb[:], ONES[:], ivrow[:], start=True, stop=True)
            nc.vector.scalar_tensor_tensor(
                pl_big[:, b * t : (b + 1) * t],
                pl_raw[:, b * t : (b + 1) * t],
                float(EPS),
                psb[:],
                OP.add,
                OP.mult,
            )

        # ---------------- phase B: windowed wavefront --------------------
        E0 = pers.tile([bs, 1 + t], F32)
        Ebuf = [pers.tile([bs, 1 + t], F32, name=f"Eb{i}") for i in range(2)]
        Zbuf = [pers.tile([bs, 1 + t], F32, name=f"Zb{i}") for i in range(3)]
        Dbuf = [pers.tile([bs, t], F32, name=f"Db{i}") for i in range(2)]
        plbuf = [pers.tile([bs, t], F32, name=f"plb{i}") for i in range(6)]
        nc.gpsimd.memset(E0[:, 0:1], 1.0)
        nc.gpsimd.memset(Zbuf[0][:, 0:1], 0.0)

        def shuffle(k, dst):
            nc.sync.dma_start(dst[:], pl_big[k : k + 1, :])

        # lane 0: windows [0, W+1]
        h0 = W + 1  # 387
        nc.vector.tensor_tensor_scan(
            E0[:, 1 : 1 + h0], PBS[:, 0:h0], ZERO[:, 0:h0], E0[:, 0:1], OP.mult, OP.add
        )
        shuffle(0, plbuf[0])
        nc.vector.tensor_tensor_scan(
            Zbuf[0][:, 1 : 1 + h0],
            E0[:, 1 : 1 + h0],
            plbuf[0][:, 0:h0],
            Zbuf[0][:, 0:1],
            OP.add,
            OP.mult,
        )
        prevZ = Zbuf[0]
        for k in range(1, l):
            Ek = Ebuf[k % 2]
            Zk = Zbuf[k % 3]
            plk = plbuf[k % 6]
            shuffle(k, plk)
            lo = k - 1
            hiE = min(t, lo + W + 1)  # E window end (time index)
            hiZ = min(t, k + W)  # z window end
            nc.gpsimd.memset(Ek[:, lo : lo + 1], 0.0)
            nc.gpsimd.memset(Zk[:, k : k + 1], 0.0)
            nc.vector.tensor_tensor_scan(
                Ek[:, lo + 1 : hiE + 1],
                PBS[:, lo:hiE],
                prevZ[:, lo:hiE],
                Ek[:, lo : lo + 1],
                OP.mult,
                OP.add,
            )
            if k in slowset:
                dl = Dbuf[k % 2]
                nz = hiZ - k
                nc.vector.scalar_tensor_tensor(
                    dl[:, 0:nz],
                    prevZ[:, k:hiZ],
                    KM1[:, k : k + 1],
                    Ek[:, k + 1 : hiZ + 1],
                    OP.mult,
                    OP.add,
                )
                zin = dl[:, 0:nz]
            else:
                zin = Ek[:, k + 1 : hiZ + 1]
            nc.vector.tensor_tensor_scan(
                Zk[:, k + 1 : hiZ + 1],
                zin,
                plk[:, k:hiZ],
                Zk[:, k : k + 1],
                OP.add,
                OP.mult,
            )
            prevZ = Zk
        EL = Ebuf[l % 2]
        nc.gpsimd.memset(EL[:, l - 1 : l], 0.0)
        nc.vector.tensor_tensor_scan(
            EL[:, l : 1 + t],
            PBS[:, l - 1 : t],
            prevZ[:, l - 1 : t],
            EL[:, l - 1 : l],
            OP.mult,
            OP.add,
        )
        nc.vector.tensor_add(FIN[:], EL[:, t : t + 1], prevZ[:, t : t + 1])
        nc.scalar.activation(LLOG[:], FIN[:], AF.Ln)
        nc.vector.tensor_sub(LOSS[:], LOGACC[:], LLOG[:])
        nc.sync.dma_start(lossd[:], LOSS[:])

    nc.finalize()
    _nc_cache[key] = nc
    return nc


def lane_flags(y_true, l=L):
    """Lanes needing the kap stt: any repeated label at position k across the
    FULL batch (one SPMD program serves all cores)."""
    yt = np.asarray(y_true, dtype=np.int32)
    rep = yt[:, 1:] == yt[:, :-1]
    return tuple(int(k) for k in range(1, l) if rep[:, k - 1].any())


def host_prep(y_true, y_pred, bs=BS, t=T, c=C, l=L):
    CT = c // 128
    GW = l + bs
    ncores = y_true.shape[0] // bs
    ones = np.ones((1, 128), dtype=NPBF)
    maps = []
    for core in range(ncores):
        sl = slice(core * bs, (core + 1) * bs)
        yt = np.asarray(y_true[sl], dtype=np.int32)
        ypT = np.asarray(y_pred[sl], dtype=np.float32).transpose(0, 2, 1)
        ypk = np.ascontiguousarray(
            ypT.reshape(bs, CT, 128, t).transpose(0, 2, 1, 3).reshape(bs, 128, CT * t)
        ).astype(NPBF)
        G = (yt[:, None, :] == np.arange(c, dtype=np.int32)[None, :, None]).astype(
            np.float32
        )
        cnt = G.sum(axis=2)
        cnt[:, c - 1] = l + 1.0
        gx = np.zeros((bs, c, GW), dtype=np.float32)
        gx[:, :, :l] = G
        for b in range(bs):
            gx[b, :, l + b] = cnt[b]
        gxk = np.ascontiguousarray(
            gx.reshape(bs, CT, 128, GW).transpose(0, 2, 1, 3).reshape(bs, 128, CT * GW)
        ).astype(NPBF)
        kap = np.zeros((bs, l), dtype=np.float32)
        kap[:, 1:] = (yt[:, 1:] != yt[:, :-1]).astype(np.float32)
        maps.append({"ypk": ypk, "gxk": gxk, "km1": kap - 1.0, "ones": ones})
    return maps


def kernel(y_true, y_pred):
    nc = build_nc(lane_flags(y_true))
    maps = host_prep(y_true, y_pred)
    res = run_bass_kernel_spmd(nc, maps, list(range(NCORES)))
    loss = np.concatenate([res.results[i]["loss"] for i in range(NCORES)], axis=0)
    return loss.astype(np.float32)


# revision 20
# speedup vs baseline: 1.5878x; 1.0200x over previous
"""CTC loss (keras ctc_batch_cost semantics) on 8 Trainium2 NeuronCores.

Linear-space CTC forward DP as a wavefront over extended-label lanes: for
each label lane k the whole time axis is one hardware linear-recurrence
instruction (tensor_tensor_scan on the DVE), so the serial chain is over k
(128 steps), not t (512 steps).

Gauge-transformed recurrences (z_k = O_k * psi, psi_j = pb'_j, psi_T = 1):

  E[k]_j = pb'_j * E[k]_{j-1} + z[k-1]_{j-1}          (scan: mult-add)
  z[k]_j = (E[k]_j + z[k]_{j-1}) * Q[k]_{j-1}         (scan: add-mult)

with Q[k]_j = pl'[k]_j * pb'_{j+1} / pb'_j.  The skip-forbidden correction
(repeated labels) adds (kap-1)*z[k-1]_{j-1} inside the z-scan's in0; lanes
where kap==1 for every batch row skip that stt -- 2 DVE ops per lane.

Windowed scans: lane k's alpha is 0 for t < k (not enough emissions) and
irrelevant for t > T-L+1+k (cannot reach the final states), so each scan
covers a 386..387-step window instead of 512 -- exact, not approximate.

Probabilities are pre-scaled by invr_t = s1_t/s2_t (gauge, cancels exactly
in the loss; sum_t log r_t is added back).  Phase A runs in bf16; phase B
in fp32 (bf16 is slower on the DVE scans).
"""

import sys

for _p in ("/opt/trn_rl_repo",):
    if _p not in sys.path:
        sys.path.insert(0, _p)

from contextlib import ExitStack

import ml_dtypes
import numpy as np

import concourse.bacc as bacc
import concourse.bass as bass
import concourse.tile as tile
from concourse import mybir
from concourse.bass_utils import run_bass_kernel_spmd

F32 = mybir.dt.float32
BF16 = mybir.dt.bfloat16
AF = mybir.ActivationFunctionType
OP = mybir.AluOpType
NPBF = np.dtype(ml_dtypes.bfloat16)

B, T, C, L = 256, 512, 256, 128
NCORES = 8
BS = B // NCORES
EPS = 1e-7
BLANK = C - 1

_nc_cache = {}


def build_nc(slow, bs=BS, t=T, c=C, l=L):
    """slow: tuple of lane indices (1..l-1) needing the kap correction stt."""
    key = (tuple(slow), bs, t, c, l)
    if key in _nc_cache:
        return _nc_cache[key]
    slowset = set(slow)
    CT = c // 128
    GW = l + bs
    W = t - l + 2  # scan window length (386)
    nc = bacc.Bacc("TRN2")
    ypk = nc.declare_dram_parameter("ypk", [bs, 128, CT * t], BF16, isOutput=False)
    gxk = nc.declare_dram_parameter("gxk", [bs, 128, CT * GW], BF16, isOutput=False)
    km1d = nc.declare_dram_parameter("km1", [bs, l], F32, isOutput=False)
    onesd = nc.declare_dram_parameter("ones", [1, 128], BF16, isOutput=False)
    lossd = nc.declare_dram_parameter("loss", [bs, 1], F32, isOutput=True)

    with ExitStack() as ctx:
        tc = ctx.enter_context(tile.TileContext(nc))
        pers = ctx.enter_context(tc.tile_pool(name="pers", bufs=1))
        ypool = ctx.enter_context(tc.tile_pool(name="y", bufs=3))
        y2pool = ctx.enter_context(tc.tile_pool(name="y2", bufs=3))
        gpool = ctx.enter_context(tc.tile_pool(name="g", bufs=3))
        rowpool = ctx.enter_context(tc.tile_pool(name="row", bufs=3))
        pspool = ctx.enter_context(
            tc.tile_pool(name="ps", bufs=3, space=bass.MemorySpace.PSUM)
        )
        psacc = ctx.enter_context(
            tc.tile_pool(name="psacc", bufs=1, space=bass.MemorySpace.PSUM)
        )
        psbc = ctx.enter_context(
            tc.tile_pool(name="psbc", bufs=2, space=bass.MemorySpace.PSUM)
        )

        pl_raw = pers.tile([128, bs * t], BF16)
        pl_big = pers.tile([128, bs * t], F32)
        PB = pers.tile([bs, t], BF16)
        PBe = pers.tile([bs, t], F32)
        PBR = pers.tile([bs, t], F32)
        PBSH = pers.tile([bs, t], F32)
        PRE1 = pers.tile([bs, t], F32)
        INVSH = pers.tile([bs, t], F32)
        PBS = pers.tile([bs, t], F32)
        RFAC = pers.tile([bs, t], F32)
        RFAC16 = pers.tile([bs, t], BF16)
        INVR = pers.tile([bs, t], F32)
        IV = pers.tile([bs, t], F32)
        SCR = pers.tile([bs, t], F32)
        KM1 = pers.tile([bs, l], F32)
        ONES = pers.tile([1, 128], BF16)
        LOGACC = pers.tile([bs, 1], F32)
        ZERO = pers.tile([bs, t], F32)
        FIN = pers.tile([bs, 1], F32)
        LLOG = pers.tile([bs, 1], F32)
        LOSS = pers.tile([bs, 1], F32)

        nc.sync.dma_start(KM1[:], km1d[:])
        nc.sync.dma_start(ONES[:], onesd[:])
        nc.gpsimd.memset(ZERO[:], 0.0)

        ps_s1 = psacc.tile([bs, t], F32)
        ps_s2 = psacc.tile([bs, t], F32)

        # ---------------- phase A: gather + s1/s2 ------------------------
        for b in range(bs):
            y = ypool.tile([128, CT * t], BF16, tag="y")
            nc.sync.dma_start(y[:], ypk[b])
            g = gpool.tile([128, CT * GW], BF16, tag="g")
            nc.gpsimd.dma_start(g[:], gxk[b])
            nc.gpsimd.dma_start(PB[b : b + 1, :], ypk[b, 127:128, t : 2 * t])
            y2 = y2pool.tile([128, CT * t], BF16, tag="y2")
            nc.scalar.activation(y2[:], y[:], AF.Square)
            psg = pspool.tile([128, t], F32, tag="plps")
            for ci in range(CT):
                nc.tensor.matmul(
                    psg[0:l, :],
                    g[:, ci * GW : ci * GW + l],
                    y[:, ci * t : (ci + 1) * t],
                    start=(ci == 0),
                    stop=(ci == CT - 1),
                )
            for ci in range(CT):
                nc.tensor.matmul(
                    ps_s1[:],
                    g[:, ci * GW + l : (ci + 1) * GW],
                    y[:, ci * t : (ci + 1) * t],
                    start=(b == 0 and ci == 0),
                    stop=(b == bs - 1 and ci == CT - 1),
                )
            for ci in range(CT):
                nc.tensor.matmul(
                    ps_s2[:],
                    g[:, ci * GW + l : (ci + 1) * GW],
                    y2[:, ci * t : (ci + 1) * t],
                    start=(b == 0 and ci == 0),
                    stop=(b == bs - 1 and ci == CT - 1),
                )
            nc.scalar.copy(pl_raw[:, b * t : (b + 1) * t], psg[0:l, :])

        # pb-derived gauge factors (PB rows complete mid-loop; overlaps A)
        nc.vector.scalar_tensor_tensor(
            PBe[:], PB[:], float(EPS), ZERO[:], OP.add, OP.add
        )
        nc.vector.reciprocal(PBR[:], PBe[:])
        nc.scalar.copy(PBSH[:, 0 : t - 1], PBe[:, 1:t])
        nc.gpsimd.memset(PBSH[:, t - 1 : t], 1.0)
        nc.vector.tensor_mul(PRE1[:], PBSH[:], PBR[:])

        # ---------------- tail: invr, gauge factors, scaled probs --------
        nc.vector.reciprocal(IV[:], ps_s2[:])
        nc.vector.tensor_mul(INVR[:], IV[:], ps_s1[:])
        nc.vector.tensor_mul(PBS[:], PBe[:], INVR[:])
        nc.scalar.activation(SCR[:], INVR[:], AF.Ln, accum_out=LOGACC[:])
        nc.scalar.copy(INVSH[:, 0 : t - 1], INVR[:, 1:t])
        nc.gpsimd.memset(INVSH[:, t - 1 : t], 1.0)
        nc.vector.tensor_mul(RFAC[:], PRE1[:], INVSH[:])
        nc.scalar.copy(RFAC16[:], RFAC[:])
        for b in range(bs):
            ivrow = rowpool.tile([1, t], BF16, tag="ivrow")
            nc.gpsimd.dma_start(ivrow[:], RFAC16[b : b + 1, :])
            psb = psbc.tile([128, t], F32, tag="bc")
            nc.tensor.matmul(psb[:], ONES[:], ivrow[:], start=True, stop=True)
            nc.vector.scalar_tensor_tensor(
                pl_big[:, b * t : (b + 1) * t],
                pl_raw[:, b * t : (b + 1) * t],
                float(EPS),
                psb[:],
                OP.add,
                OP.mult,
            )

        # ---------------- phase B: windowed wavefront --------------------
        E0 = pers.tile([bs, 1 + t], F32)
        Ebuf = [pers.tile([bs, 1 + t], F32, name=f"Eb{i}") for i in range(2)]
        Zbuf = [pers.tile([bs, 1 + t], F32, name=f"Zb{i}") for i in range(3)]
        Dbuf = [pers.tile([bs, t], F32, name=f"Db{i}") for i in range(2)]
        plbuf = [pers.tile([bs, t], F32, name=f"plb{i}") for i in range(6)]
        nc.gpsimd.memset(E0[:, 0:1], 1.0)
        nc.gpsimd.memset(Zbuf[0][:, 0:1], 0.0)

        def shuffle(k, dst):
            nc.sync.dma_start(dst[:], pl_big[k : k + 1, :])

        # lane 0: windows [0, W+1]
        h0 = W + 1  # 387
        nc.vector.tensor_tensor_scan(
            E0[:, 1 : 1 + h0], PBS[:, 0:h0], ZERO[:, 0:h0], E0[:, 0:1], OP.mult, OP.add
        )
        shuffle(0, plbuf[0])
        nc.vector.tensor_tensor_scan(
            Zbuf[0][:, 1 : 1 + h0],
            E0[:, 1 : 1 + h0],
            plbuf[0][:, 0:h0],
            Zbuf[0][:, 0:1],
            OP.add,
            OP.mult,
        )
        prevZ = Zbuf[0]
        for k in range(1, l):
            Ek = Ebuf[k % 2]
            Zk = Zbuf[k % 3]
            plk = plbuf[k % 6]
            shuffle(k, plk)
            lo = k - 1
            hiE = min(t, lo + W + 1)  # E window end (time index)
            hiZ = min(t, k + W)  # z window end
            nc.gpsimd.memset(Ek[:, lo : lo + 1], 0.0)
            nc.gpsimd.memset(Zk[:, k : k + 1], 0.0)
            nc.vector.tensor_tensor_scan(
                Ek[:, lo + 1 : hiE + 1],
                PBS[:, lo:hiE],
                prevZ[:, lo:hiE],
                Ek[:, lo : lo + 1],
                OP.mult,
                OP.add,
            )
            if k in slowset:
                dl = Dbuf[k % 2]
                nz = hiZ - k
                nc.vector.scalar_tensor_tensor(
                    dl[:, 0:nz],
                    prevZ[:, k:hiZ],
                    KM1[:, k : k + 1],
                    Ek[:, k + 1 : hiZ + 1],
                    OP.mult,
                    OP.add,
                )
                zin = dl[:, 0:nz]
            else:
                zin = Ek[:, k + 1 : hiZ + 1]
            nc.vector.tensor_tensor_scan(
                Zk[:, k + 1 : hiZ + 1],
                zin,
                plk[:, k:hiZ],
                Zk[:, k : k + 1],
                OP.add,
                OP.mult,
            )
            prevZ = Zk
        EL = Ebuf[l % 2]
        nc.gpsimd.memset(EL[:, l - 1 : l], 0.0)
        nc.vector.tensor_tensor_scan(
            EL[:, l : 1 + t],
            PBS[:, l - 1 : t],
            prevZ[:, l - 1 : t],
            EL[:, l - 1 : l],
            OP.mult,
            OP.add,
        )
        nc.vector.tensor_add(FIN[:], EL[:, t : t + 1], prevZ[:, t : t + 1])
        nc.scalar.activation(LLOG[:], FIN[:], AF.Ln)
        nc.vector.tensor_sub(LOSS[:], LOGACC[:], LLOG[:])
        nc.sync.dma_start(lossd[:], LOSS[:])

    nc.finalize()
    _nc_cache[key] = nc
    return nc


def lane_flags(y_true, l=L):
    """Lanes needing the kap stt: any repeated label at position k across the
    FULL batch (one SPMD program serves all cores)."""
    yt = np.asarray(y_true, dtype=np.int32)
    rep = yt[:, 1:] == yt[:, :-1]
    return tuple(int(k) for k in range(1, l) if rep[:, k - 1].any())


def host_prep(y_true, y_pred, bs=BS, t=T, c=C, l=L):
    CT = c // 128
    GW = l + bs
    ncores = y_true.shape[0] // bs
    ones = np.ones((1, 128), dtype=NPBF)
    maps = []
    for core in range(ncores):
        sl = slice(core * bs, (core + 1) * bs)
        yt = np.asarray(y_true[sl], dtype=np.int32)
        ypT = np.asarray(y_pred[sl], dtype=np.float32).transpose(0, 2, 1)
        ypk = np.ascontiguousarray(
            ypT.reshape(bs, CT, 128, t).transpose(0, 2, 1, 3).reshape(bs, 128, CT * t)
        ).astype(NPBF)
        G = (yt[:, None, :] == np.arange(c, dtype=np.int32)[None, :, None]).astype(
            np.float32
        )
        cnt = G.sum(axis=2)
        cnt[:, c - 1] = l + 1.0
        gx = np.zeros((bs, c, GW), dtype=np.float32)
        gx[:, :, :l] = G
        for b in range(bs):
            gx[b, :, l + b] = cnt[b]
        gxk = np.ascontiguousarray(
            gx.reshape(bs, CT, 128, GW).transpose(0, 2, 1, 3).reshape(bs, 128, CT * GW)
        ).astype(NPBF)
        kap = np.zeros((bs, l), dtype=np.float32)
        kap[:, 1:] = (yt[:, 1:] != yt[:, :-1]).astype(np.float32)
        maps.append({"ypk": ypk, "gxk": gxk, "km1": kap - 1.0, "ones": ones})
    return maps


def kernel(y_true, y_pred):
    nc = build_nc(lane_flags(y_true))
    maps = host_prep(y_true, y_pred)
    res = run_bass_kernel_spmd(nc, maps, list(range(NCORES)))
    loss = np.concatenate([res.results[i]["loss"] for i in range(NCORES)], axis=0)
    return loss.astype(np.float32)
